# revision 10
# baseline (speedup 1.0000x reference)
"""Trainium2 Bass kernel for ContrastiveLoss (N=16384, D=1024, 8 NeuronCores).

Strategy (data-parallel over anchors):
  - Host shards rows across 8 cores: core i owns anchor rows [2048*i, 2048*(i+1)).
  - Host gathers pos/neg rows (gather commutes with row-wise normalization) and
    converts to fp16, so each core receives three contiguous [2048, 1024] fp16
    blocks (halves HBM traffic; fp16 keeps ~1e-5 relative accuracy here).
  - Device computes, per row r: sum(u*u), sum(u*v), sum(u*w) with a
    triple-buffered raw-Bass pipeline:
      ScalarE: Square+accum (row norm^2), Copy+accum (reduce of u*v product)
      VectorE: tensor_tensor mult fp16 2x mode (u*v, u*w), tensor_reduce (u*w)
      SP:      1MB HWDGE DMA loads, stats store
  - Row norms of pos/neg rows are gathers of the same global norm array, so
    the host epilogue (f64) reconstructs the reference math:
      ||a-b+eps||^2 = |a|^2 + |b|^2 + D*eps^2 - 2<a,b> (+ O(eps) sum terms,
      dropped: ~1e-8 relative), a = u/max(|u|,eps), then the margin loss.
"""

import sys

for _p in ("/opt/trn_rl_repo", "/root/.axon_site/_ro/trn_rl_repo"):
    if _p not in sys.path:
        sys.path.append(_p)

import numpy as np

N = 16384  # total rows
D = 1024  # embedding dim
NCORES = 8
RPC = N // NCORES  # rows per core = 2048
T = RPC // 128  # row-tiles per core = 16
G = 2  # row-tiles per DMA group (512 KB fp16 per load)
NG = T // G  # DMA groups per core = 4
BUFS = 5  # in-flight groups
EPS = 1e-6
MARGIN = 1.0

LAST_RESULT = None
_CACHE = {}


def _build_nc():
    import concourse.bass as bass
    import concourse.mybir as mybir

    f32 = mybir.dt.float32
    f16 = mybir.dt.float16
    nc = bass.Bass()
    anc = nc.declare_dram_parameter("anc", [RPC, D], f16, isOutput=False)
    pos = nc.declare_dram_parameter("pos", [RPC, D], f16, isOutput=False)
    neg = nc.declare_dram_parameter("neg", [RPC, D], f16, isOutput=False)
    out = nc.declare_dram_parameter("out", [3, 128, T], f32, isOutput=True)

    # DRAM row-tile t holds rows [128*t, 128*t+128); G tiles per DMA group.
    anc_r = anc[:, :].rearrange("(g a p) d -> g p a d", p=128, a=G)
    pos_r = pos[:, :].rearrange("(g a p) d -> g p a d", p=128, a=G)
    neg_r = neg[:, :].rearrange("(g a p) d -> g p a d", p=128, a=G)
    out_ap = out[:, :, :]

    Sq = mybir.ActivationFunctionType.Square
    Cp = mybir.ActivationFunctionType.Copy
    mult = mybir.AluOpType.mult
    add = mybir.AluOpType.add
    AX = mybir.AxisListType.X

    # dotp reduce owner: every 3rd sub-tile on DVE, rest on ACT (load balance)
    def on_dve(t):
        return t % 3 == 0

    def cnt_redp(k):  # of DVE-owned dotp reduces among sub-tiles 0..k
        return sum(1 for j in range(k + 1) if on_dve(j))

    def cnt_cp(k):  # of ACT-owned dotp reduces among sub-tiles 0..k
        return sum(1 for j in range(k + 1) if not on_dve(j))

    NCP_TOT = cnt_cp(T - 1)
    NREDP_TOT = cnt_redp(T - 1)

    from contextlib import ExitStack

    with ExitStack() as ctx:
        sb = lambda nm, shape, dt: ctx.enter_context(nc.sbuf_tensor(nm, shape, dt))
        ps = lambda nm, shape, dt: ctx.enter_context(nc.psum_tensor(nm, shape, dt))
        sem = lambda nm: ctx.enter_context(nc.semaphore(nm))

        U = [sb(f"u{i}", [128, G, D], f16) for i in range(BUFS)]
        V = [sb(f"v{i}", [128, G, D], f16) for i in range(BUFS)]
        W = [sb(f"w{i}", [128, G, D], f16) for i in range(BUFS)]
        SQD = [ps(f"sqd{i}", [128, D], f32) for i in range(2)]  # ACT Square dumps
        CPD = [ps(f"cpd{i}", [128, D], f32) for i in range(2)]  # ACT Copy dumps
        S2 = [sb(f"s2{i}", [128, D], f16) for i in range(3)]  # DVE u*v product
        S3 = [sb(f"s3{i}", [128, D], f16) for i in range(3)]  # GPS u*w product
        nu2 = sb("nu2", [128, T], f32)
        dotp = sb("dotp", [128, T], f32)
        dotn = sb("dotn", [128, T], f32)
        # per-(tensor, slot) load sems: at most one outstanding DMA each,
        # so completion order is unambiguous
        SEM_U = [sem(f"sem_u{i}") for i in range(BUFS)]
        SEM_V = [sem(f"sem_v{i}") for i in range(BUFS)]
        SEM_W = [sem(f"sem_w{i}") for i in range(BUFS)]
        st_sem = sem("st_sem")  # +16 per completed store DMA
        # per-op-class retirement sems (count = ops retired); these give the
        # race detector an explicit edge for every buffer reuse
        dve_s2 = sem("dve_s2")  # DVE TT (u*v -> S2)
        gps_s3 = sem("gps_s3")  # GPS TT (u*w -> S3)
        dve_red = sem("dve_red")  # DVE reduce (S3 -> dotn col)
        dve_redp = sem("dve_redp")  # DVE reduce (S2 -> dotp col, t%3==0)
        act_sq = sem("act_sq")  # ACT Square (u -> nu2 col)
        act_s2 = sem("act_s2")  # ACT Copy (S2 -> dotp col, t%3!=0)
        block = ctx.enter_context(nc.Block())

        @block.sync
        def _(sync):
            for g in range(NG):
                if g >= BUFS:
                    m = G * (g - BUFS + 1)  # consumers of slot g-BUFS retired
                    sync.wait_ge(dve_s2, m)  # TT1 reads of U,V
                    sync.wait_ge(gps_s3, m)  # TT2 reads of U,W
                    sync.wait_ge(act_sq, m)  # Square reads of U
                b = g % BUFS
                sync.dma_start(out=U[b][:], in_=anc_r[g]).then_inc(SEM_U[b], 16)
                sync.dma_start(out=V[b][:], in_=pos_r[g]).then_inc(SEM_V[b], 16)
                sync.dma_start(out=W[b][:], in_=neg_r[g]).then_inc(SEM_W[b], 16)
            sync.wait_ge(act_sq, T)
            sync.wait_ge(dve_red, T)
            sync.wait_ge(act_s2, NCP_TOT)
            sync.wait_ge(dve_redp, NREDP_TOT)
            sync.dma_start(out=out_ap[0], in_=nu2[:]).then_inc(st_sem, 16)
            sync.dma_start(out=out_ap[1], in_=dotp[:]).then_inc(st_sem, 16)
            sync.dma_start(out=out_ap[2], in_=dotn[:]).then_inc(st_sem, 16)
            sync.wait_ge(st_sem, 48)

        @block.vector
        def _(vector):
            def reduces(t):
                # reduces for sub-tile t, issued one sub-tile late so the
                # producing TTs retired long before (no pipeline stall)
                vector.wait_ge(gps_s3, t + 1)
                nc.vector.tensor_reduce(
                    out=dotn[:, t : t + 1], in_=S3[t % 3][:], axis=AX, op=add
                ).then_inc(dve_red, 1)
                if on_dve(t):
                    vector.wait_ge(dve_s2, t + 1)
                    nc.vector.tensor_reduce(
                        out=dotp[:, t : t + 1], in_=S2[t % 3][:], axis=AX, op=add
                    ).then_inc(dve_redp, 1)

            for g in range(NG):
                b = g % BUFS
                k = 16 * (g // BUFS + 1)
                vector.wait_ge(SEM_U[b], k)
                vector.wait_ge(SEM_V[b], k)
                for a in range(G):
                    t = g * G + a
                    if t >= 3:  # S2 slot: consumer of t-3 retired
                        if on_dve(t - 3):
                            vector.wait_ge(dve_redp, cnt_redp(t - 3))
                        else:
                            vector.wait_ge(act_s2, cnt_cp(t - 3))
                    nc.vector.tensor_tensor(
                        out=S2[t % 3][:], in0=U[b][:, a, :], in1=V[b][:, a, :],
                        op=mult,
                    ).then_inc(dve_s2, 1)
                    if t >= 1:
                        reduces(t - 1)
            reduces(T - 1)

        @block.gpsimd
        def _(gpsimd):
            for g in range(NG):
                b = g % BUFS
                k = 16 * (g // BUFS + 1)
                gpsimd.wait_ge(SEM_U[b], k)
                gpsimd.wait_ge(SEM_W[b], k)
                for a in range(G):
                    t = g * G + a
                    if t >= 3:  # S3 slot: reduce of t-3 retired
                        gpsimd.wait_ge(dve_red, t - 2)
                    nc.gpsimd.tensor_tensor(
                        out=S3[t % 3][:], in0=U[b][:, a, :], in1=W[b][:, a, :],
                        op=mult,
                    ).then_inc(gps_s3, 1)

        @block.scalar
        def _(scalar):
            def cp(t):
                # ACT-owned dotp reduce for sub-tile t (issued one late)
                c = cnt_cp(t)  # 1-based count including t
                scalar.wait_ge(dve_s2, t + 1)  # product retired
                if c >= 3:
                    scalar.wait_ge(act_s2, c - 2)  # CPD slot writer retired
                nc.scalar.activation(
                    out=CPD[(c - 1) % 2][:], in_=S2[t % 3][:], func=Cp,
                    accum_out=dotp[:, t : t + 1],
                ).then_inc(act_s2, 1)

            for g in range(NG):
                b = g % BUFS
                scalar.wait_ge(SEM_U[b], 16 * (g // BUFS + 1))  # u loaded
                for a in range(G):
                    t = g * G + a
                    if t >= 2:
                        scalar.wait_ge(act_sq, t - 1)  # SQD slot writer retired
                    nc.scalar.activation(
                        out=SQD[t % 2][:], in_=U[b][:, a, :], func=Sq,
                        accum_out=nu2[:, t : t + 1],
                    ).then_inc(act_sq, 1)
                    if t >= 1 and not on_dve(t - 1):
                        cp(t - 1)
            if not on_dve(T - 1):
                cp(T - 1)

    return nc


def kernel(embeddings, labels, pos_idx, neg_idx):
    global LAST_RESULT
    from concourse.bass_utils import run_bass_kernel_spmd

    emb = np.asarray(embeddings, dtype=np.float32).astype(np.float16)
    assert emb.shape == (N, D)
    pidx = np.asarray(pos_idx).astype(np.int64)
    nidx = np.asarray(neg_idx).astype(np.int64)

    in_maps = []
    for i in range(NCORES):
        sl = slice(i * RPC, (i + 1) * RPC)
        in_maps.append(
            {
                "anc": np.ascontiguousarray(emb[sl]),
                "pos": np.ascontiguousarray(emb[pidx[sl]]),
                "neg": np.ascontiguousarray(emb[nidx[sl]]),
            }
        )

    nc = _CACHE.get("nc")
    if nc is None:
        nc = _build_nc()
        _CACHE["nc"] = nc

    res = run_bass_kernel_spmd(nc, in_maps, list(range(NCORES)))
    LAST_RESULT = res

    # out[k] is [128, T]: row p, col t -> shard row t*128+p
    def decode(k):
        return np.concatenate(
            [res.results[i]["out"][k].T.ravel() for i in range(NCORES)]
        ).astype(np.float64)

    nu2 = decode(0)
    P = decode(1)
    Q = decode(2)

    norm = np.sqrt(nu2)
    den = np.maximum(norm, EPS)  # F.normalize clamp
    ahat2 = nu2 / (den * den)  # ||a_hat||^2 (==1 unless degenerate)

    def dist(idx, dot):
        S = ahat2 + ahat2[idx] - 2.0 * dot / (den * den[idx]) + D * EPS * EPS
        return np.sqrt(np.maximum(S, 0.0)) + EPS

    d_pos = dist(pidx, P)
    d_neg = dist(nidx, Q)
    pos_loss = d_pos * d_pos
    neg_loss = np.maximum(MARGIN - d_neg, EPS) ** 2
    total = pos_loss.sum() + neg_loss.sum()
    return np.array(total / (2.0 * N), dtype=np.float32)


# revision 11
# speedup vs baseline: 1.0884x; 1.0884x over previous
"""Trainium2 Bass kernel for ContrastiveLoss (N=16384, D=1024, 8 NeuronCores).

Strategy (data-parallel over anchors):
  - Host shards rows across 8 cores: core i owns anchor rows [2048*i, 2048*(i+1)).
  - Host gathers pos/neg rows (gather commutes with row-wise normalization) and
    converts to fp16, so each core receives three contiguous [2048, 1024] fp16
    blocks (halves HBM traffic; fp16 keeps ~1e-5 relative accuracy here).
  - Device computes, per row r: sum(u*u), sum(u*v), sum(u*w) with a
    triple-buffered raw-Bass pipeline:
      ScalarE: Square+accum (row norm^2), Copy+accum (reduce of u*v product)
      VectorE: tensor_tensor mult fp16 2x mode (u*v, u*w), tensor_reduce (u*w)
      SP:      1MB HWDGE DMA loads, stats store
  - Row norms of pos/neg rows are gathers of the same global norm array, so
    the host epilogue (f64) reconstructs the reference math:
      ||a-b+eps||^2 = |a|^2 + |b|^2 + D*eps^2 - 2<a,b> (+ O(eps) sum terms,
      dropped: ~1e-8 relative), a = u/max(|u|,eps), then the margin loss.
"""

import sys

for _p in ("/opt/trn_rl_repo", "/root/.axon_site/_ro/trn_rl_repo"):
    if _p not in sys.path:
        sys.path.append(_p)

import numpy as np

N = 16384  # total rows
D = 1024  # embedding dim
NCORES = 8
RPC = N // NCORES  # rows per core = 2048
T = RPC // 128  # row-tiles per core = 16
G = 2  # row-tiles per DMA group (512 KB fp16 per load)
NG = T // G  # DMA groups per core = 4
BUFS = 5  # in-flight groups
EPS = 1e-6
MARGIN = 1.0

LAST_RESULT = None
_CACHE = {}


def _build_nc():
    import concourse.bass as bass
    import concourse.mybir as mybir

    f32 = mybir.dt.float32
    f16 = mybir.dt.float16
    nc = bass.Bass()
    anc = nc.declare_dram_parameter("anc", [RPC, D], f16, isOutput=False)
    pos = nc.declare_dram_parameter("pos", [RPC, D], f16, isOutput=False)
    neg = nc.declare_dram_parameter("neg", [RPC, D], f16, isOutput=False)
    out = nc.declare_dram_parameter("out", [3, 128, T], f32, isOutput=True)

    # DRAM row-tile t holds rows [128*t, 128*t+128); G tiles per DMA group.
    anc_r = anc[:, :].rearrange("(g a p) d -> g p a d", p=128, a=G)
    pos_r = pos[:, :].rearrange("(g a p) d -> g p a d", p=128, a=G)
    neg_r = neg[:, :].rearrange("(g a p) d -> g p a d", p=128, a=G)
    out_ap = out[:, :, :]

    Sq = mybir.ActivationFunctionType.Square
    Cp = mybir.ActivationFunctionType.Copy
    mult = mybir.AluOpType.mult
    add = mybir.AluOpType.add
    AX = mybir.AxisListType.X

    from contextlib import ExitStack

    with ExitStack() as ctx:
        sb = lambda nm, shape, dt: ctx.enter_context(nc.sbuf_tensor(nm, shape, dt))
        ps = lambda nm, shape, dt: ctx.enter_context(nc.psum_tensor(nm, shape, dt))
        sem = lambda nm: ctx.enter_context(nc.semaphore(nm))

        U = [sb(f"u{i}", [128, G, D], f16) for i in range(BUFS)]
        V = [sb(f"v{i}", [128, G, D], f16) for i in range(BUFS)]
        W = [sb(f"w{i}", [128, G, D], f16) for i in range(BUFS)]
        SQD = [ps(f"sqd{i}", [128, D], f32) for i in range(2)]  # ACT Square dumps
        CPD = [ps(f"cpd{i}", [128, D], f32) for i in range(2)]  # ACT Copy dumps
        S2 = [sb(f"s2{i}", [128, D], f16) for i in range(3)]  # DVE u*v product
        S3 = [sb(f"s3{i}", [128, D], f16) for i in range(3)]  # DVE u*w product
        nu2 = sb("nu2", [128, T], f32)
        dotp = sb("dotp", [128, T], f32)
        dotn = sb("dotn", [128, T], f32)
        # per-(tensor, slot) load sems: at most one outstanding DMA each,
        # so completion order is unambiguous
        SEM_U = [sem(f"sem_u{i}") for i in range(BUFS)]
        SEM_V = [sem(f"sem_v{i}") for i in range(BUFS)]
        SEM_W = [sem(f"sem_w{i}") for i in range(BUFS)]
        st_sem = sem("st_sem")  # +16 per completed store DMA
        # per-op-class retirement sems (count = ops retired); these give the
        # race detector an explicit edge for every buffer reuse
        dve_s2 = sem("dve_s2")  # DVE TT (u*v -> S2)
        dve_s3 = sem("dve_s3")  # DVE TT (u*w -> S3)
        dve_red = sem("dve_red")  # DVE reduce (S3 -> dotn col)
        act_sq = sem("act_sq")  # ACT Square (u -> nu2 col)
        act_s2 = sem("act_s2")  # ACT Copy (S2 -> dotp col, t%3!=0)
        block = ctx.enter_context(nc.Block())

        @block.sync
        def _(sync):
            for g in range(NG):
                if g >= BUFS:
                    m = G * (g - BUFS + 1)  # consumers of slot g-BUFS retired
                    sync.wait_ge(dve_s2, m)  # TT1 reads of U,V
                    sync.wait_ge(dve_s3, m)  # TT2 reads of U,W
                    sync.wait_ge(act_sq, m)  # Square reads of U
                b = g % BUFS
                sync.dma_start(out=U[b][:], in_=anc_r[g]).then_inc(SEM_U[b], 16)
                sync.dma_start(out=V[b][:], in_=pos_r[g]).then_inc(SEM_V[b], 16)
                sync.dma_start(out=W[b][:], in_=neg_r[g]).then_inc(SEM_W[b], 16)
            sync.wait_ge(act_sq, T)
            sync.wait_ge(dve_red, T)
            sync.wait_ge(act_s2, T)
            sync.dma_start(out=out_ap[0], in_=nu2[:]).then_inc(st_sem, 16)
            sync.dma_start(out=out_ap[1], in_=dotp[:]).then_inc(st_sem, 16)
            sync.dma_start(out=out_ap[2], in_=dotn[:]).then_inc(st_sem, 16)
            sync.wait_ge(st_sem, 48)

        @block.vector
        def _(vector):
            def reduces(t):
                # reduces for sub-tile t, issued one sub-tile late so the
                # producing TTs retired long before (no pipeline stall)
                vector.wait_ge(dve_s3, t + 1)
                nc.vector.tensor_reduce(
                    out=dotn[:, t : t + 1], in_=S3[t % 3][:], axis=AX, op=add
                ).then_inc(dve_red, 1)

            for g in range(NG):
                b = g % BUFS
                k = 16 * (g // BUFS + 1)
                vector.wait_ge(SEM_U[b], k)
                vector.wait_ge(SEM_V[b], k)
                for a in range(G):
                    t = g * G + a
                    if t >= 3:  # S2 slot: ACT copy of t-3 retired
                        vector.wait_ge(act_s2, t - 2)
                    nc.vector.tensor_tensor(
                        out=S2[t % 3][:], in0=U[b][:, a, :], in1=V[b][:, a, :],
                        op=mult,
                    ).then_inc(dve_s2, 1)
                    if a == 0:
                        vector.wait_ge(SEM_W[b], k)  # w loaded
                    if t >= 3:  # S3 slot: reduce of t-3 retired
                        vector.wait_ge(dve_red, t - 2)
                    nc.vector.tensor_tensor(
                        out=S3[t % 3][:], in0=U[b][:, a, :], in1=W[b][:, a, :],
                        op=mult,
                    ).then_inc(dve_s3, 1)
                    if t >= 1:
                        reduces(t - 1)
            reduces(T - 1)

        @block.scalar
        def _(scalar):
            def cp(t):
                # dotp reduce for sub-tile t (issued one sub-tile late)
                scalar.wait_ge(dve_s2, t + 1)  # product retired
                if t >= 2:
                    scalar.wait_ge(act_s2, t - 1)  # CPD slot writer retired
                nc.scalar.activation(
                    out=CPD[t % 2][:], in_=S2[t % 3][:], func=Cp,
                    accum_out=dotp[:, t : t + 1],
                ).then_inc(act_s2, 1)

            for g in range(NG):
                b = g % BUFS
                scalar.wait_ge(SEM_U[b], 16 * (g // BUFS + 1))  # u loaded
                for a in range(G):
                    t = g * G + a
                    if t >= 2:
                        scalar.wait_ge(act_sq, t - 1)  # SQD slot writer retired
                    nc.scalar.activation(
                        out=SQD[t % 2][:], in_=U[b][:, a, :], func=Sq,
                        accum_out=nu2[:, t : t + 1],
                    ).then_inc(act_sq, 1)
                    if t >= 1:
                        cp(t - 1)
            cp(T - 1)

    return nc


def kernel(embeddings, labels, pos_idx, neg_idx):
    global LAST_RESULT
    from concourse.bass_utils import run_bass_kernel_spmd

    emb = np.asarray(embeddings, dtype=np.float32).astype(np.float16)
    assert emb.shape == (N, D)
    pidx = np.asarray(pos_idx).astype(np.int64)
    nidx = np.asarray(neg_idx).astype(np.int64)

    in_maps = []
    for i in range(NCORES):
        sl = slice(i * RPC, (i + 1) * RPC)
        in_maps.append(
            {
                "anc": np.ascontiguousarray(emb[sl]),
                "pos": np.ascontiguousarray(emb[pidx[sl]]),
                "neg": np.ascontiguousarray(emb[nidx[sl]]),
            }
        )

    nc = _CACHE.get("nc")
    if nc is None:
        nc = _build_nc()
        _CACHE["nc"] = nc

    res = run_bass_kernel_spmd(nc, in_maps, list(range(NCORES)))
    LAST_RESULT = res

    # out[k] is [128, T]: row p, col t -> shard row t*128+p
    def decode(k):
        return np.concatenate(
            [res.results[i]["out"][k].T.ravel() for i in range(NCORES)]
        ).astype(np.float64)

    nu2 = decode(0)
    P = decode(1)
    Q = decode(2)

    norm = np.sqrt(nu2)
    den = np.maximum(norm, EPS)  # F.normalize clamp
    ahat2 = nu2 / (den * den)  # ||a_hat||^2 (==1 unless degenerate)

    def dist(idx, dot):
        S = ahat2 + ahat2[idx] - 2.0 * dot / (den * den[idx]) + D * EPS * EPS
        return np.sqrt(np.maximum(S, 0.0)) + EPS

    d_pos = dist(pidx, P)
    d_neg = dist(nidx, Q)
    pos_loss = d_pos * d_pos
    neg_loss = np.maximum(MARGIN - d_neg, EPS) ** 2
    total = pos_loss.sum() + neg_loss.sum()
    return np.array(total / (2.0 * N), dtype=np.float32)
